# revision 20
# baseline (speedup 1.0000x reference)
"""dX-privacy embedding snap (argmax over vocab of noisy-embedding scores)
for Trainium2, 8 NeuronCores.

Distribution: vocab-sharded. Core c owns rows [c*4000, (c+1)*4000) of the
embedding table; every core scores all 8192 tokens against its shard.

Host prep (layout only): x = inputs_embeds + noise; x^T in [32,128,T] fp16;
per-core E^T shard tiled [32 k][8 vb][128 d][500 v] fp16 (offline-style
weight layout).

Device (per core, all FLOPs): for each 1024-token block, keep x^T resident in
SBUF, stream E^T tiles, fp16 matmuls (128x128x500) accumulating fp32 scores
[token_tile, v] in PSUM over the 32 k-tiles; VectorEngine max/max_index emit
per-(token, v-block) top-8 values + in-block indices, written out densely.

Host finish: merge the 8*8*8 = 512 candidates per token, take the argmax;
tokens whose top-2 margin < THETA are rescored with a float32 einsum over the
full vocab (mirrors the reference arithmetic); output = embed_table[winner].

fp16 rounding gives score error sigma ~= 0.036 on D=4096 dot products vs a
mean top-1/top-2 gap of ~20. Candidates within THETA=0.5 of the top always
survive into the host merge (P(miss) ~ 1e-22), so the final argmax matches a
float32 reference exactly.
"""

import sys, os, time

sys.path.insert(0, "/opt/trn_rl_repo")
import numpy as np

import bass_rust
import concourse.bass as bass
import concourse.mybir as mybir
from concourse import tile


f32 = mybir.dt.float32
f32r = mybir.dt.float32r
f16 = mybir.dt.float16
u32 = mybir.dt.uint32

B, S, D, V = 4, 2048, 4096, 32000
T = B * S  # 8192 tokens
N_CORES = 8
VSH = V // N_CORES  # 4000 vocab rows per core (no padding needed)
KT = D // 128  # 32 k tiles
NVB = 8  # v blocks per core
NV = 500  # v block width (8*500 = 4000)
NTB = 8  # t blocks
TB = T // NTB  # 1024 tokens per t block
NTT = TB // 128  # 8 t tiles per block
THETA = 0.5  # host rescore margin

_mwfix_ctr = [0]


def _legalize_multiwaits(nc, max_waits=1):
    """walrus encodes at most one sem wait per instruction; split multi-wait
    instructions by inserting single-wait NOPs before them (same engine)."""
    for fn in nc.m.functions:
        for bb in fn.blocks:
            insts = list(bb.instructions)
            out = []
            changed = False
            for inst in insts:
                si = inst.sync_info
                ow = list(si.on_wait) if si is not None and si.on_wait else []
                if len(ow) > max_waits:
                    for wentry in ow[:-max_waits]:
                        _mwfix_ctr[0] += 1
                        nop = mybir.InstNoOp(
                            name=f"mwfix-{_mwfix_ctr[0]}", ins=[], outs=[]
                        )
                        nop.engine = inst.engine
                        nop.sync_info = bass_rust.SyncInfo(
                            on_wait=[wentry], on_update=[]
                        )
                        out.append(nop)
                    si.on_wait = ow[-max_waits:]
                    changed = True
                out.append(inst)
            if changed:
                bb.instructions = out


def _build_nc():
    nc = bass.Bass()
    xt_in = nc.declare_dram_parameter("xt", [KT, 128, T], f16, isOutput=False)
    et_in = nc.declare_dram_parameter("et", [KT, NVB, 128, NV], f16, isOutput=False)
    out_val = nc.declare_dram_parameter(
        "val8", [NTB * NVB, 128, NTT * 8], f32, isOutput=True
    )
    out_idx = nc.declare_dram_parameter(
        "idx8", [NTB * NVB, 128, NTT * 8], u32, isOutput=True
    )

    with tile.TileContext(nc) as tc:
        with (
            tc.tile_pool(name="xt", bufs=2) as xtp,
            tc.tile_pool(name="et", bufs=20) as etp,
            tc.tile_pool(name="o8", bufs=2) as o8p,
            tc.tile_pool(name="bnc", bufs=2) as bncp,
            tc.tile_pool(name="ps", bufs=1, space="PSUM") as ps,
        ):
            for tb in range(NTB):
                # B1: load this t-block's x^T slice (host pre-transposed).
                # Double-buffered slots (bufs=2) + the separate ACT HWDGE
                # queue let t-block tb+1's loads prefetch during tb's
                # compute instead of serializing at the block boundary.
                xt_tiles = []
                for k in range(KT):
                    t = xtp.tile([128, TB], f16, tag=f"xt{k}", name=f"xt_{tb}_{k}")
                    nc.scalar.dma_start(
                        t[:], xt_in[k, :, tb * TB : (tb + 1) * TB]
                    )
                    xt_tiles.append(t)

                # B2: stream E^T tiles, matmul, fold top-8 per (t-tile, vblock)
                for vb in range(NVB):
                    psums = []
                    for tt in range(NTT):
                        pst = ps.tile(
                            [128, NV], f32, tag=f"ps{tt}", name=f"ps_{tb}_{vb}_{tt}"
                        )
                        psums.append(pst)
                    for k in range(KT):
                        et_sb = etp.tile(
                            [128, NV], f16, tag="et_sb", name=f"et_{tb}_{vb}_{k}"
                        )
                        nc.sync.dma_start(et_sb[:], et_in[k, vb])
                        for tt in range(NTT):
                            nc.tensor.matmul(
                                psums[tt][:],
                                xt_tiles[k][:, tt * 128 : (tt + 1) * 128],
                                et_sb[:],
                                start=(k == 0),
                                stop=(k == KT - 1),
                            )
                    val8 = o8p.tile([128, NTT * 8], f32, tag="val8", name=f"v8_{tb}_{vb}")
                    idx8 = o8p.tile([128, NTT * 8], u32, tag="idx8", name=f"i8_{tb}_{vb}")
                    for tt in range(NTT):
                        # bounce out of PSUM first so the bank recycles for the
                        # next v-block; max/max_index then run off-critical-path
                        bsc = bncp.tile([128, NV], f32, tag=f"bn{tt % 4}", name=f"bn_{tb}_{vb}_{tt}")
                        nc.vector.tensor_copy(bsc[:], psums[tt][:])
                        nc.vector.max(out=val8[:, tt * 8 : (tt + 1) * 8], in_=bsc[:])
                        nc.vector.max_index(
                            out=idx8[:, tt * 8 : (tt + 1) * 8],
                            in_max=val8[:, tt * 8 : (tt + 1) * 8],
                            in_values=bsc[:],
                        )
                    nc.sync.dma_start(out_val[tb * NVB + vb], val8[:])
                    nc.sync.dma_start(out_idx[tb * NVB + vb], idx8[:])
    _legalize_multiwaits(nc)
    return nc


_RUNNER = None
LAST_TIMES = None  # per-call wall times of the timed iterations


def _get_runner():
    global _RUNNER
    if _RUNNER is not None:
        return _RUNNER
    import jax
    from jax.sharding import Mesh, PartitionSpec, NamedSharding
    from jax.experimental.shard_map import shard_map
    from concourse.bass2jax import (
        _bass_exec_p,
        install_neuronx_cc_hook,
        partition_id_tensor,
    )

    nc = _build_nc()
    install_neuronx_cc_hook()
    partition_name = nc.partition_id_tensor.name if nc.partition_id_tensor else None

    in_names, out_names, out_avals, zero_outs = [], [], [], []
    for alloc in nc.m.functions[0].allocations:
        if not isinstance(alloc, mybir.MemoryLocationSet):
            continue
        name = alloc.memorylocations[0].name
        if alloc.kind == "ExternalInput":
            if name != partition_name:
                in_names.append(name)
        elif alloc.kind == "ExternalOutput":
            shape, dt = alloc.tensor_shape, mybir.dt.np(alloc.dtype)
            out_names.append(name)
            out_avals.append(jax.core.ShapedArray(shape, dt))
            zero_outs.append(np.zeros(shape, dt))

    n_params = len(in_names)
    all_in_names = list(in_names) + list(out_names)
    if partition_name is not None:
        all_in_names.append(partition_name)

    def _body(*args):
        operands = list(args)
        if partition_name is not None:
            operands.append(partition_id_tensor())
        outs = _bass_exec_p.bind(
            *operands,
            out_avals=tuple(out_avals),
            in_names=tuple(all_in_names),
            out_names=tuple(out_names),
            lowering_input_output_aliases=(),
            sim_require_finite=True,
            sim_require_nnan=True,
            nc=nc,
        )
        return tuple(outs)

    devices = jax.devices()[:N_CORES]
    mesh = Mesh(np.asarray(devices), ("core",))
    in_specs = (PartitionSpec("core"),) * (n_params + len(out_names))
    out_specs = (PartitionSpec("core"),) * len(out_names)
    fn = jax.jit(
        shard_map(
            _body, mesh=mesh, in_specs=in_specs, out_specs=out_specs, check_rep=False
        ),
        keep_unused=True,
    )

    def run(in_maps, n_iters=1):
        global LAST_TIMES
        args = []
        for name in in_names:
            shards = [
                jax.device_put(np.ascontiguousarray(in_maps[c][name]), devices[c])
                for c in range(N_CORES)
            ]
            per_shape = shards[0].shape
            gshape = (N_CORES * per_shape[0],) + tuple(per_shape[1:])
            args.append(
                jax.make_array_from_single_device_arrays(
                    gshape, NamedSharding(mesh, PartitionSpec("core")), shards
                )
            )
        zargs = []
        for z in zero_outs:
            shards = [jax.device_put(z, d) for d in devices]
            gshape = (N_CORES * z.shape[0],) + tuple(z.shape[1:])
            zargs.append(
                jax.make_array_from_single_device_arrays(
                    gshape, NamedSharding(mesh, PartitionSpec("core")), shards
                )
            )
        out = fn(*args, *zargs)
        jax.block_until_ready(out)
        globals()["_FN"] = fn
        globals()["_ARGS"] = (args, zargs)
        times = []
        for _ in range(n_iters - 1):
            t0 = time.perf_counter()
            out = fn(*args, *zargs)
            jax.block_until_ready(out)
            times.append(time.perf_counter() - t0)
        LAST_TIMES = times
        results = []
        for c in range(N_CORES):
            m = {}
            for i, name in enumerate(out_names):
                ga = np.asarray(out[i]).reshape((N_CORES,) + out_avals[i].shape)
                m[name] = ga[c]
            results.append(m)
        return results

    _RUNNER = run
    return run


def measure_exec_ns(chains=(20, 120), tries=3):
    """Per-execution device time via long-chain slope (cancels the tunnel's
    ~40ms completion-poll quantization). Requires a prior kernel() call."""
    import jax

    fn = globals().get("_FN")
    args, zargs = globals().get("_ARGS")
    best = None
    for _ in range(tries):
        ts = []
        for n in chains:
            o = fn(*args, *zargs)
            jax.block_until_ready(o)  # sync point
            t0 = time.perf_counter()
            for _ in range(n):
                o = fn(*args, *zargs)
            jax.block_until_ready(o)
            ts.append(time.perf_counter() - t0)
        per = (ts[1] - ts[0]) / (chains[1] - chains[0])
        best = per if best is None else min(best, per)
    return best * 1e9


def kernel(inputs_embeds, embed_table, noise):
    verbose = os.environ.get("KERNEL_VERBOSE")
    _t = [time.time()]

    def _lap(msg):
        if verbose:
            t = time.time()
            print(f"[kernel] {msg}: {t - _t[0]:.1f}s", flush=True)
            _t[0] = t

    inputs_embeds = np.asarray(inputs_embeds)
    embed_table = np.asarray(embed_table)
    noise = np.asarray(noise)

    # host prep
    x = (inputs_embeds + noise).reshape(T, D).astype(np.float32)
    xt = np.ascontiguousarray(x.T.astype(np.float16)).reshape(KT, 128, T)
    in_maps = []
    for c in range(N_CORES):
        sh = embed_table[c * VSH : (c + 1) * VSH]  # [4000 v, 4096 d]
        et = np.ascontiguousarray(
            sh.reshape(NVB, NV, KT, 128).transpose(2, 0, 3, 1).astype(np.float16)
        )  # [32 k, 8 vb, 128 d, 500 v] fp16
        in_maps.append({"xt": xt, "et": et})
    _lap("host prep")

    run = _get_runner()
    _lap("compile/runner")
    n_iters = int(os.environ.get("KERNEL_TIME_ITERS", "1"))
    results = run(in_maps, n_iters=n_iters)
    _lap("stage+run")

    # host merge: candidates [T, 8 cores * 8 vb * 8] -> global argmax
    cand_vals = np.empty((T, N_CORES * NVB * 8), dtype=np.float32)
    cand_idx = np.empty((T, N_CORES * NVB * 8), dtype=np.int64)
    for c in range(N_CORES):
        # [tb*NVB+vb, 128 p, tt*8+e] -> token (tb*NTT+tt)*128+p, cand (vb, e)
        v8 = results[c]["val8"].reshape(NTB, NVB, 128, NTT, 8)
        i8 = results[c]["idx8"].astype(np.int64).reshape(NTB, NVB, 128, NTT, 8)
        v8 = v8.transpose(0, 3, 2, 1, 4).reshape(T, NVB * 8)
        i8 = i8.transpose(0, 3, 2, 1, 4).reshape(T, NVB * 8)
        vb_off = (np.arange(NVB * 8) // 8) * NV
        gi = c * VSH + vb_off[None, :] + i8
        cand_vals[:, c * NVB * 8 : (c + 1) * NVB * 8] = v8
        cand_idx[:, c * NVB * 8 : (c + 1) * NVB * 8] = gi

    order = np.argsort(cand_vals, axis=1)[:, ::-1]
    best = order[:, 0]
    second = order[:, 1]
    rows = np.arange(T)
    win_idx = cand_idx[rows, best]
    margin = cand_vals[rows, best] - cand_vals[rows, second]

    # safety net: exact (reference-style fp32) rescore of low-margin tokens
    flagged = np.where(margin < THETA)[0]
    if flagged.size:
        import jax.numpy as jnp
        import jax as _jax

        with _jax.default_device(_jax.devices("cpu")[0]):
            s = jnp.einsum(
                "td,vd->tv",
                jnp.asarray(x[flagged]),
                jnp.asarray(embed_table),
            )
            win_idx[flagged] = np.asarray(jnp.argmax(s, axis=-1))

    _lap(f"merge+rescore ({flagged.size} flagged)")
    out = embed_table[win_idx].reshape(B, S, D)
    _lap("gather")
    return out


# revision 21
# speedup vs baseline: 1.0230x; 1.0230x over previous
"""dX-privacy embedding snap (argmax over vocab of noisy-embedding scores)
for Trainium2, 8 NeuronCores.

Distribution: vocab-sharded. Core c owns rows [c*4000, (c+1)*4000) of the
embedding table; every core scores all 8192 tokens against its shard.

Host prep (layout only): x = inputs_embeds + noise; x^T in [32,128,T] fp16;
per-core E^T shard tiled [32 k][8 vb][128 d][500 v] fp16 (offline-style
weight layout).

Device (per core, all FLOPs): for each 1024-token block, keep x^T resident in
SBUF, stream E^T tiles, fp16 matmuls (128x128x500) accumulating fp32 scores
[token_tile, v] in PSUM over the 32 k-tiles; VectorEngine max/max_index emit
per-(token, v-block) top-8 values + in-block indices, written out densely.

Host finish: merge the 8*8*8 = 512 candidates per token, take the argmax;
tokens whose top-2 margin < THETA are rescored with a float32 einsum over the
full vocab (mirrors the reference arithmetic); output = embed_table[winner].

fp16 rounding gives score error sigma ~= 0.036 on D=4096 dot products vs a
mean top-1/top-2 gap of ~20. Candidates within THETA=0.5 of the top always
survive into the host merge (P(miss) ~ 1e-22), so the final argmax matches a
float32 reference exactly.
"""

import sys, os, time

sys.path.insert(0, "/opt/trn_rl_repo")
import numpy as np

import bass_rust
import concourse.bass as bass
import concourse.mybir as mybir
from concourse import tile


f32 = mybir.dt.float32
f32r = mybir.dt.float32r
f16 = mybir.dt.float16
u32 = mybir.dt.uint32

B, S, D, V = 4, 2048, 4096, 32000
T = B * S  # 8192 tokens
N_CORES = 8
VSH = V // N_CORES  # 4000 vocab rows per core (no padding needed)
KT = D // 128  # 32 k tiles
NVB = 8  # v blocks per core
NV = 500  # v block width (8*500 = 4000)
NTB = 8  # t blocks
TB = T // NTB  # 1024 tokens per t block
NTT = TB // 128  # 8 t tiles per block
THETA = 0.5  # host rescore margin

_mwfix_ctr = [0]


def _legalize_multiwaits(nc, max_waits=1):
    """walrus encodes at most one sem wait per instruction; split multi-wait
    instructions by inserting single-wait NOPs before them (same engine)."""
    for fn in nc.m.functions:
        for bb in fn.blocks:
            insts = list(bb.instructions)
            out = []
            changed = False
            for inst in insts:
                si = inst.sync_info
                ow = list(si.on_wait) if si is not None and si.on_wait else []
                if len(ow) > max_waits:
                    for wentry in ow[:-max_waits]:
                        _mwfix_ctr[0] += 1
                        nop = mybir.InstNoOp(
                            name=f"mwfix-{_mwfix_ctr[0]}", ins=[], outs=[]
                        )
                        nop.engine = inst.engine
                        nop.sync_info = bass_rust.SyncInfo(
                            on_wait=[wentry], on_update=[]
                        )
                        out.append(nop)
                    si.on_wait = ow[-max_waits:]
                    changed = True
                out.append(inst)
            if changed:
                bb.instructions = out


def _build_nc():
    nc = bass.Bass()
    xt_in = nc.declare_dram_parameter("xt", [KT, 128, T], f16, isOutput=False)
    et_in = nc.declare_dram_parameter("et", [KT, NVB, 128, NV], f16, isOutput=False)
    out_val = nc.declare_dram_parameter(
        "val8", [NTB * NVB, 128, NTT * 8], f32, isOutput=True
    )
    out_idx = nc.declare_dram_parameter(
        "idx8", [NTB * NVB, 128, NTT * 8], u32, isOutput=True
    )

    with tile.TileContext(nc) as tc:
        with (
            tc.tile_pool(name="xt", bufs=2) as xtp,
            tc.tile_pool(name="et", bufs=20) as etp,
            tc.tile_pool(name="o8", bufs=2) as o8p,
            tc.tile_pool(name="bnc", bufs=2) as bncp,
            tc.tile_pool(name="ps", bufs=1, space="PSUM") as ps,
        ):
            for tb in range(NTB):
                # B1: load this t-block's x^T slice (host pre-transposed).
                # Double-buffered slots (bufs=2) + the separate ACT HWDGE
                # queue let t-block tb+1's loads prefetch during tb's
                # compute instead of serializing at the block boundary.
                xt_tiles = []
                for k in range(KT):
                    t = xtp.tile([128, TB], f16, tag=f"xt{k}", name=f"xt_{tb}_{k}")
                    nc.scalar.dma_start(
                        t[:], xt_in[k, :, tb * TB : (tb + 1) * TB]
                    )
                    xt_tiles.append(t)

                # B2: stream E^T tiles, matmul, fold top-8 per (t-tile, vblock)
                for vb in range(NVB):
                    psums = []
                    for tt in range(NTT):
                        pst = ps.tile(
                            [128, NV], f32, tag=f"ps{tt}", name=f"ps_{tb}_{vb}_{tt}"
                        )
                        psums.append(pst)
                    for k in range(KT):
                        et_sb = etp.tile(
                            [128, NV], f16, tag="et_sb", name=f"et_{tb}_{vb}_{k}"
                        )
                        nc.sync.dma_start(et_sb[:], et_in[k, vb])
                        for tt in range(NTT):
                            nc.tensor.matmul(
                                psums[tt][:],
                                xt_tiles[k][:, tt * 128 : (tt + 1) * 128],
                                et_sb[:],
                                start=(k == 0),
                                stop=(k == KT - 1),
                            )
                    val8 = o8p.tile([128, NTT * 8], f32, tag="val8", name=f"v8_{tb}_{vb}")
                    idx8 = o8p.tile([128, NTT * 8], u32, tag="idx8", name=f"i8_{tb}_{vb}")
                    for tt in range(NTT):
                        # bounce out of PSUM first so the bank recycles for the
                        # next v-block; max/max_index then run off-critical-path
                        bsc = bncp.tile([128, NV], f32, tag=f"bn{tt % 4}", name=f"bn_{tb}_{vb}_{tt}")
                        nc.vector.tensor_copy(bsc[:], psums[tt][:])
                        nc.vector.max(out=val8[:, tt * 8 : (tt + 1) * 8], in_=bsc[:])
                        nc.vector.max_index(
                            out=idx8[:, tt * 8 : (tt + 1) * 8],
                            in_max=val8[:, tt * 8 : (tt + 1) * 8],
                            in_values=bsc[:],
                        )
                    # ACT queue: keep result writes out of the E^T load FIFO
                    nc.scalar.dma_start(out_val[tb * NVB + vb], val8[:])
                    nc.scalar.dma_start(out_idx[tb * NVB + vb], idx8[:])
    _legalize_multiwaits(nc)
    return nc


_RUNNER = None
LAST_TIMES = None  # per-call wall times of the timed iterations


def _get_runner():
    global _RUNNER
    if _RUNNER is not None:
        return _RUNNER
    import jax
    from jax.sharding import Mesh, PartitionSpec, NamedSharding
    from jax.experimental.shard_map import shard_map
    from concourse.bass2jax import (
        _bass_exec_p,
        install_neuronx_cc_hook,
        partition_id_tensor,
    )

    nc = _build_nc()
    install_neuronx_cc_hook()
    partition_name = nc.partition_id_tensor.name if nc.partition_id_tensor else None

    in_names, out_names, out_avals, zero_outs = [], [], [], []
    for alloc in nc.m.functions[0].allocations:
        if not isinstance(alloc, mybir.MemoryLocationSet):
            continue
        name = alloc.memorylocations[0].name
        if alloc.kind == "ExternalInput":
            if name != partition_name:
                in_names.append(name)
        elif alloc.kind == "ExternalOutput":
            shape, dt = alloc.tensor_shape, mybir.dt.np(alloc.dtype)
            out_names.append(name)
            out_avals.append(jax.core.ShapedArray(shape, dt))
            zero_outs.append(np.zeros(shape, dt))

    n_params = len(in_names)
    all_in_names = list(in_names) + list(out_names)
    if partition_name is not None:
        all_in_names.append(partition_name)

    def _body(*args):
        operands = list(args)
        if partition_name is not None:
            operands.append(partition_id_tensor())
        outs = _bass_exec_p.bind(
            *operands,
            out_avals=tuple(out_avals),
            in_names=tuple(all_in_names),
            out_names=tuple(out_names),
            lowering_input_output_aliases=(),
            sim_require_finite=True,
            sim_require_nnan=True,
            nc=nc,
        )
        return tuple(outs)

    devices = jax.devices()[:N_CORES]
    mesh = Mesh(np.asarray(devices), ("core",))
    in_specs = (PartitionSpec("core"),) * (n_params + len(out_names))
    out_specs = (PartitionSpec("core"),) * len(out_names)
    fn = jax.jit(
        shard_map(
            _body, mesh=mesh, in_specs=in_specs, out_specs=out_specs, check_rep=False
        ),
        keep_unused=True,
    )

    def run(in_maps, n_iters=1):
        global LAST_TIMES
        args = []
        for name in in_names:
            shards = [
                jax.device_put(np.ascontiguousarray(in_maps[c][name]), devices[c])
                for c in range(N_CORES)
            ]
            per_shape = shards[0].shape
            gshape = (N_CORES * per_shape[0],) + tuple(per_shape[1:])
            args.append(
                jax.make_array_from_single_device_arrays(
                    gshape, NamedSharding(mesh, PartitionSpec("core")), shards
                )
            )
        zargs = []
        for z in zero_outs:
            shards = [jax.device_put(z, d) for d in devices]
            gshape = (N_CORES * z.shape[0],) + tuple(z.shape[1:])
            zargs.append(
                jax.make_array_from_single_device_arrays(
                    gshape, NamedSharding(mesh, PartitionSpec("core")), shards
                )
            )
        out = fn(*args, *zargs)
        jax.block_until_ready(out)
        globals()["_FN"] = fn
        globals()["_ARGS"] = (args, zargs)
        times = []
        for _ in range(n_iters - 1):
            t0 = time.perf_counter()
            out = fn(*args, *zargs)
            jax.block_until_ready(out)
            times.append(time.perf_counter() - t0)
        LAST_TIMES = times
        results = []
        for c in range(N_CORES):
            m = {}
            for i, name in enumerate(out_names):
                ga = np.asarray(out[i]).reshape((N_CORES,) + out_avals[i].shape)
                m[name] = ga[c]
            results.append(m)
        return results

    _RUNNER = run
    return run


def measure_exec_ns(chains=(20, 120), tries=3):
    """Per-execution device time via long-chain slope (cancels the tunnel's
    ~40ms completion-poll quantization). Requires a prior kernel() call."""
    import jax

    fn = globals().get("_FN")
    args, zargs = globals().get("_ARGS")
    best = None
    for _ in range(tries):
        ts = []
        for n in chains:
            o = fn(*args, *zargs)
            jax.block_until_ready(o)  # sync point
            t0 = time.perf_counter()
            for _ in range(n):
                o = fn(*args, *zargs)
            jax.block_until_ready(o)
            ts.append(time.perf_counter() - t0)
        per = (ts[1] - ts[0]) / (chains[1] - chains[0])
        best = per if best is None else min(best, per)
    return best * 1e9


def kernel(inputs_embeds, embed_table, noise):
    verbose = os.environ.get("KERNEL_VERBOSE")
    _t = [time.time()]

    def _lap(msg):
        if verbose:
            t = time.time()
            print(f"[kernel] {msg}: {t - _t[0]:.1f}s", flush=True)
            _t[0] = t

    inputs_embeds = np.asarray(inputs_embeds)
    embed_table = np.asarray(embed_table)
    noise = np.asarray(noise)

    # host prep
    x = (inputs_embeds + noise).reshape(T, D).astype(np.float32)
    xt = np.ascontiguousarray(x.T.astype(np.float16)).reshape(KT, 128, T)
    in_maps = []
    for c in range(N_CORES):
        sh = embed_table[c * VSH : (c + 1) * VSH]  # [4000 v, 4096 d]
        et = np.ascontiguousarray(
            sh.reshape(NVB, NV, KT, 128).transpose(2, 0, 3, 1).astype(np.float16)
        )  # [32 k, 8 vb, 128 d, 500 v] fp16
        in_maps.append({"xt": xt, "et": et})
    _lap("host prep")

    run = _get_runner()
    _lap("compile/runner")
    n_iters = int(os.environ.get("KERNEL_TIME_ITERS", "1"))
    results = run(in_maps, n_iters=n_iters)
    _lap("stage+run")

    # host merge: candidates [T, 8 cores * 8 vb * 8] -> global argmax
    cand_vals = np.empty((T, N_CORES * NVB * 8), dtype=np.float32)
    cand_idx = np.empty((T, N_CORES * NVB * 8), dtype=np.int64)
    for c in range(N_CORES):
        # [tb*NVB+vb, 128 p, tt*8+e] -> token (tb*NTT+tt)*128+p, cand (vb, e)
        v8 = results[c]["val8"].reshape(NTB, NVB, 128, NTT, 8)
        i8 = results[c]["idx8"].astype(np.int64).reshape(NTB, NVB, 128, NTT, 8)
        v8 = v8.transpose(0, 3, 2, 1, 4).reshape(T, NVB * 8)
        i8 = i8.transpose(0, 3, 2, 1, 4).reshape(T, NVB * 8)
        vb_off = (np.arange(NVB * 8) // 8) * NV
        gi = c * VSH + vb_off[None, :] + i8
        cand_vals[:, c * NVB * 8 : (c + 1) * NVB * 8] = v8
        cand_idx[:, c * NVB * 8 : (c + 1) * NVB * 8] = gi

    order = np.argsort(cand_vals, axis=1)[:, ::-1]
    best = order[:, 0]
    second = order[:, 1]
    rows = np.arange(T)
    win_idx = cand_idx[rows, best]
    margin = cand_vals[rows, best] - cand_vals[rows, second]

    # safety net: exact (reference-style fp32) rescore of low-margin tokens
    flagged = np.where(margin < THETA)[0]
    if flagged.size:
        import jax.numpy as jnp
        import jax as _jax

        with _jax.default_device(_jax.devices("cpu")[0]):
            s = jnp.einsum(
                "td,vd->tv",
                jnp.asarray(x[flagged]),
                jnp.asarray(embed_table),
            )
            win_idx[flagged] = np.asarray(jnp.argmax(s, axis=-1))

    _lap(f"merge+rescore ({flagged.size} flagged)")
    out = embed_table[win_idx].reshape(B, S, D)
    _lap("gather")
    return out
